# revision 39
# baseline (speedup 1.0000x reference)
"""Trainium2 Bass kernel for factorized space-time attention.

Computation (per batch b of 8, one NeuronCore each):
  qkv = x @ w_qkv.T                      (3136, 2304)
  heads 0-5:  spatial attention over 196 patches within each of 16 frames
  heads 6-11: temporal attention over groups of 16 consecutive tokens
              (raw-reshape semantics of the reference)
  out = concat(head outputs) @ w_proj.T + b_proj

Strategy: data-parallel over batch (8 cores). All activations kept
feature-major ([d, n]) on chip so every matmul contraction runs over the
partition dim with no on-device transposes; x / weights are pre-transposed
host-side. V is produced token-major directly by flipping the projection
matmul orientation, with a ones column appended so the AV matmul emits the
softmax denominator in psum row 64 for free.

The kernel is software-pipelined at emission level: the QKV/V projection
matmuls for superblock s+1 are interleaved between the attention groups of
superblock s, so the in-order PE always has independent projection work
while Act/DVE process softmax (exp, reciprocal, normalize).  Softmax
normalization is batched: per group only reciprocals (into per-head-pair
collector rows) and raw-AV copies run; per superblock-half the collected
reciprocals are partition-broadcast by ones-matmuls and applied with wide
[64,392] muls, and head1 rows reach attnT partitions 64:128 via batched
DMA. Temporal masking runs on the otherwise idle Pool (gpsimd) engine.

All matmul operands are bf16 (4x faster than fp32 on the PE); accumulation
stays fp32 in PSUM.
"""

import sys

if "/opt/trn_rl_repo" not in sys.path:
    sys.path.append("/opt/trn_rl_repo")

import numpy as np

import concourse.bass as bass  # noqa: F401
import concourse.mybir as mybir
import concourse.tile as tile
from concourse import bacc
from concourse.bass_utils import run_bass_kernel_spmd

F32 = mybir.dt.float32
BF16 = mybir.dt.bfloat16
AF = mybir.ActivationFunctionType

B = 8
F = 16
P = 196
D = 768
NH = 12
HD = 64
N = F * P
E3 = 3 * D
SB = 784
NSB = N // SB
FPSB = SB // P
WPSB = SB // 112
SCALE = HD ** -0.5

COMPUTE = "bf16"

_CACHE = {}


def _build(compute: str, reps: int = 1):
    cdt = BF16 if compute == "bf16" else F32

    wb = 2

    nc = bacc.Bacc("TRN2", target_bir_lowering=False, debug=False, num_devices=B)

    xt_d = nc.dram_tensor("xt", (D, N), cdt, kind="ExternalInput")
    wqkv_d = nc.dram_tensor("wqkvT", (D, E3), cdt, kind="ExternalInput")
    wproj_d = nc.dram_tensor("wprojT", (D, D), cdt, kind="ExternalInput")
    bias_d = nc.dram_tensor("bias", (D, 1), F32, kind="ExternalInput")
    mask_d = nc.dram_tensor("mask", (112, 112), cdt, kind="ExternalInput")
    out_d = nc.dram_tensor("outT", (D, N), F32, kind="ExternalOutput")

    with tile.TileContext(nc) as tc:
        with (
            tc.tile_pool(name="const", bufs=1) as cpool,
            tc.tile_pool(name="work", bufs=1) as wpool,
            tc.tile_pool(name="small", bufs=4) as spool,
            tc.tile_pool(name="psum", bufs=2, space="PSUM") as ppool,
        ):
            wq = []
            for dc in range(6):
                t = cpool.tile([128, E3], cdt, tag=f"wq{dc}", name=f"wq{dc}")
                nc.sync.dma_start(t[:], wqkv_d.ap()[128 * dc : 128 * (dc + 1), :])
                wq.append(t)
            wp = []
            for dc in range(6):
                t = cpool.tile([128, D], cdt, tag=f"wp{dc}", name=f"wp{dc}")
                nc.sync.dma_start(t[:], wproj_d.ap()[128 * dc : 128 * (dc + 1), :])
                wp.append(t)
            bias_t = cpool.tile([128, 6], F32, tag="bias", name="bias_t")
            nc.sync.dma_start(
                bias_t[:], bias_d.ap().rearrange("(e p) one -> p (e one)", p=128)
            )
            mask2_t = cpool.tile([112, 224], cdt, tag="mask", name="mask2_t")
            nc.sync.dma_start(mask2_t[:, 0:112], mask_d.ap())
            nc.sync.dma_start(mask2_t[:, 112:224], mask_d.ap())
            zeros_col = cpool.tile([128, 1], F32, tag="zeros_c", name="zeros_col")
            nc.gpsimd.memset(zeros_col[:], 0.0)
            ones64 = cpool.tile([65, 64], cdt, tag="ones64", name="ones64")
            nc.gpsimd.memset(ones64[:], 1.0)

            cp_ctr = [0]

            def bulk_copy(dst, src):
                i = cp_ctr[0] % 2
                cp_ctr[0] += 1
                if i == 0:
                    nc.scalar.copy(dst, src)
                else:
                    nc.vector.tensor_copy(dst, src)

            # ---- per-superblock tile groups ----
            def alloc_sb(s):
                d = {"vs": {}, "vt": {}}
                d["xts"] = [
                    wpool.tile([128, SB], cdt, tag=f"xts{dc}", bufs=wb,
                               name=f"xts{dc}_{s}")
                    for dc in range(6)
                ]
                d["qkvt"] = [
                    wpool.tile([128, SB], cdt, tag=f"qkvt{ti}", bufs=wb,
                               name=f"qkvt{ti}_{s}")
                    for ti in range(12)
                ]
                d["attnT"] = [
                    wpool.tile([128, SB], cdt, tag=f"attnT{i}", bufs=wb,
                               name=f"attnT{i}_{s}")
                    for i in range(6)
                ]
                d["rcol"] = [
                    wpool.tile([65, 2 * SB], cdt, tag=f"rcol{i}", bufs=wb,
                               name=f"rcol{i}_{s}")
                    for i in range(6)
                ]
                d["hst"] = [
                    wpool.tile([64, 2 * SB], cdt, tag=f"hst{i}", bufs=1,
                               name=f"hst{i}_{s}")
                    for i in range(6)
                ]
                return d

            def emit_x_dma(s, d):
                so = SB * s
                for dc in range(6):
                    nc.sync.dma_start(
                        d["xts"][dc][:],
                        xt_d.ap()[128 * dc : 128 * (dc + 1), so : so + SB],
                    )

            def emit_qk_unit(s, d, ti, j):
                ps = ppool.tile([128, 392], F32, tag="mm", bufs=2,
                                name=f"ps_qk{s}_{ti}_{j}")
                for dc in range(6):
                    nc.tensor.matmul(
                        ps[:],
                        wq[dc][:, 128 * ti : 128 * (ti + 1)],
                        d["xts"][dc][:, 392 * j : 392 * (j + 1)],
                        start=(dc == 0),
                        stop=(dc == 5),
                    )
                bulk_copy(d["qkvt"][ti][:, 392 * j : 392 * (j + 1)], ps[:])

            def emit_v_unit(s, d, kind, idx):
                if kind == "s":
                    f, ci = divmod(idx, 2)
                    m0, msz = ((0, 128), (128, 68))[ci]
                    tok0, wcol0 = 196 * f + m0, 1536
                    vname = f"vs{idx}_{s}"
                else:
                    msz, tok0, wcol0 = 112, 112 * idx, 1920
                    vname = f"vt{idx}_{s}"
                vt_ = wpool.tile([msz, 390], cdt, tag=f"v{kind}{idx}", bufs=wb,
                                 name=vname)
                ps = ppool.tile([msz, 384], F32, tag="mm", bufs=2,
                                name=f"ps_{vname}")
                for dc in range(6):
                    nc.tensor.matmul(
                        ps[:],
                        d["xts"][dc][:, tok0 : tok0 + msz],
                        wq[dc][:, wcol0 : wcol0 + 384],
                        start=(dc == 0),
                        stop=(dc == 5),
                    )
                bulk_copy(
                    vt_.rearrange("p (h c) -> p h c", c=65)[:, :, 0:64],
                    ps.rearrange("p (h c) -> p h c", c=64),
                )
                nc.gpsimd.memset(
                    vt_.rearrange("p (h c) -> p h c", c=65)[:, :, 64:65], 1.0
                )
                d["vs" if kind == "s" else "vt"][idx] = vt_

            def emit_proj_units(s, d):
                units = [lambda s=s, d=d: emit_x_dma(s, d)]
                for ti in range(12):
                    for j in range(2):
                        units.append(
                            lambda s=s, d=d, ti=ti, j=j: emit_qk_unit(s, d, ti, j)
                        )
                for idx in range(2 * FPSB):
                    units.append(
                        lambda s=s, d=d, idx=idx: emit_v_unit(s, d, "s", idx)
                    )
                for idx in range(WPSB):
                    units.append(
                        lambda s=s, d=d, idx=idx: emit_v_unit(s, d, "t", idx)
                    )
                return units

            def emit_spatial_group(s, d, f, hp, drip=None):
                fo = 196 * f
                qkvt, vs = d["qkvt"], d["vs"]
                qtile, ktile = qkvt[hp], qkvt[6 + hp]
                es_all = []
                for hi in range(2):
                    pb = 64 * hi
                    es = []
                    for ci, (m0, msz) in enumerate(((0, 128), (128, 68))):
                        ps_st = ppool.tile(
                            [msz, 196], F32, tag="st", bufs=3,
                            name=f"ps_st{s}_{f}_{hp}_{hi}_{ci}",
                        )
                        nc.tensor.matmul(
                            ps_st[:],
                            ktile[pb : pb + 64, fo + m0 : fo + m0 + msz],
                            qtile[pb : pb + 64, fo : fo + 196],
                            start=True,
                            stop=True,
                        )
                        e = spool.tile(
                            [msz, 196], cdt, tag="e", bufs=6,
                            name=f"e{s}_{f}_{hp}_{hi}_{ci}",
                        )
                        nc.scalar.activation(
                            e[:], ps_st[:], AF.Exp,
                            bias=zeros_col[:msz, :], scale=SCALE,
                        )
                        es.append(e)
                    es_all.append(es)
                if drip:
                    drip()
                av = ppool.tile([65, 392], F32, tag="av", bufs=3,
                                name=f"ps_sav{s}_{f}_{hp}")
                for hi in range(2):
                    h = 2 * hp + hi
                    for ci in range(2):
                        nc.tensor.matmul(
                            av[:, 196 * hi : 196 * hi + 196],
                            vs[2 * f + ci][:, 65 * h : 65 * h + 65],
                            es_all[hi][ci][:],
                            start=(ci == 0),
                            stop=(ci == 1),
                            skip_group_check=(hi == 1),
                        )
                with nc.allow_low_precision(reason="1/softmax-sum in cdt"):
                    nc.vector.reciprocal(
                        d["rcol"][hp][64:65].rearrange(
                            "p (h t) -> p h t", h=2
                        )[:, :, fo : fo + 196],
                        av[64:65, :].rearrange("p (h t) -> p h t", h=2),
                    )
                bulk_copy(
                    d["hst"][hp].rearrange("p (h t) -> p h t", h=2)[
                        :, :, fo : fo + 196
                    ],
                    av[0:64, :].rearrange("p (h t) -> p h t", h=2),
                )

            def emit_temporal_group(s, d, w, hp, drip=None):
                wo = 112 * w
                qkvt, vt = d["qkvt"], d["vt"]
                ems = []
                for hi in range(2):
                    pb = 64 * hi
                    ps_st = ppool.tile(
                        [112, 112], F32, tag="st", bufs=3,
                        name=f"ps_tst{s}_{w}_{hp}_{hi}",
                    )
                    nc.tensor.matmul(
                        ps_st[:],
                        qkvt[9 + hp][pb : pb + 64, wo : wo + 112],
                        qkvt[3 + hp][pb : pb + 64, wo : wo + 112],
                        start=True,
                        stop=True,
                    )
                    e = spool.tile(
                        [112, 112], cdt, tag="e", bufs=6,
                        name=f"et{s}_{w}_{hp}_{hi}",
                    )
                    nc.scalar.activation(
                        e[:], ps_st[:], AF.Exp,
                        bias=zeros_col[:112], scale=SCALE,
                    )
                    em = spool.tile(
                        [112, 112], cdt, tag="e", bufs=6,
                        name=f"em{s}_{w}_{hp}_{hi}",
                    )
                    nc.gpsimd.tensor_mul(em[:], e[:], mask2_t[:, 0:112])
                    ems.append(em)
                if drip:
                    drip()
                av = ppool.tile([65, 224], F32, tag="av", bufs=3,
                                name=f"ps_tav{s}_{w}_{hp}")
                for hi in range(2):
                    h = 6 + 2 * hp + hi
                    nc.tensor.matmul(
                        av[:, 112 * hi : 112 * hi + 112],
                        vt[w][:, 65 * (h - 6) : 65 * (h - 6) + 65],
                        ems[hi][:],
                        start=True,
                        stop=True,
                        skip_group_check=(hi == 1),
                    )
                with nc.allow_low_precision(reason="1/softmax-sum in cdt"):
                    nc.vector.reciprocal(
                        d["rcol"][3 + hp][64:65].rearrange(
                            "p (h t) -> p h t", h=2
                        )[:, :, wo : wo + 112],
                        av[64:65, :].rearrange("p (h t) -> p h t", h=2),
                    )
                bulk_copy(
                    d["hst"][3 + hp].rearrange("p (h t) -> p h t", h=2)[
                        :, :, wo : wo + 112
                    ],
                    av[0:64, :].rearrange("p (h t) -> p h t", h=2),
                )

            def emit_norm_and_out(s, d, js):
                so = SB * s
                for j in js:
                    jo = 392 * j
                    for ai in range(6):
                        rc, at = d["rcol"][ai], d["attnT"][ai]
                        rb0 = ppool.tile(
                            [64, 392], F32, tag="mm", bufs=2,
                            name=f"ps_rb0{s}_{j}_{ai}",
                        )
                        nc.tensor.matmul(
                            rb0[:], ones64[64:65, 0:64],
                            rc[64:65, jo : jo + 392],
                            start=True, stop=True,
                        )
                        nc.vector.tensor_mul(
                            at[0:64, jo : jo + 392],
                            d["hst"][ai][:, jo : jo + 392], rb0[:],
                        )
                        rb1 = ppool.tile(
                            [64, 392], F32, tag="mm", bufs=2,
                            name=f"ps_rb1{s}_{j}_{ai}",
                        )
                        nc.tensor.matmul(
                            rb1[:], ones64[64:65, 0:64],
                            rc[64:65, SB + jo : SB + jo + 392],
                            start=True, stop=True,
                        )
                        nc.vector.tensor_mul(
                            d["hst"][ai][:, jo : jo + 392],
                            d["hst"][ai][:, SB + jo : SB + jo + 392], rb1[:],
                        )
                        nc.sync.dma_start(
                            at[64:128, jo : jo + 392],
                            d["hst"][ai][:, jo : jo + 392],
                        )
                    for ec in range(6):
                        ps = ppool.tile([128, 392], F32, tag="mm", bufs=2,
                                        name=f"ps_o{s}_{ec}_{j}")
                        for dc in range(6):
                            nc.tensor.matmul(
                                ps[:],
                                wp[dc][:, 128 * ec : 128 * (ec + 1)],
                                d["attnT"][dc][:, 392 * j : 392 * (j + 1)],
                                start=(dc == 0),
                                stop=(dc == 5),
                            )
                        ot = spool.tile([128, 392], F32, tag="ot",
                                        name=f"ot{s}_{ec}_{j}")
                        nc.scalar.activation(
                            ot[:], ps[:], AF.Identity,
                            bias=bias_t[:, ec : ec + 1], scale=1.0,
                        )
                        nc.sync.dma_start(
                            out_d.ap()[
                                128 * ec : 128 * (ec + 1),
                                so + 392 * j : so + 392 * (j + 1),
                            ],
                            ot[:],
                        )

            import contextlib

            rep_ctx = tc.For_i(0, reps, 1) if reps > 1 else contextlib.nullcontext()
            with rep_ctx:
                ds = {0: alloc_sb(0)}
                for u in emit_proj_units(0, ds[0]):
                    u()
                for s in range(NSB):
                    d = ds[s]
                    # first half covers attnT cols 0:392 (frames 0-1, windows
                    # 0-3); second half the rest
                    groups0 = [("s", f, hp) for f in (0, 1) for hp in range(3)]
                    groups0 += [("t", w, hp) for w in (0, 1, 2, 3)
                                for hp in range(3)]
                    groups1 = [("s", f, hp) for f in (2, 3) for hp in range(3)]
                    groups1 += [("t", w, hp) for w in (4, 5, 6)
                                for hp in range(3)]
                    if s + 1 < NSB:
                        ds[s + 1] = alloc_sb(s + 1)
                        filler = emit_proj_units(s + 1, ds[s + 1])
                    else:
                        filler = []
                    nf = len(filler)
                    # two drip slots per group: mid-group (before AV) and
                    # after-group
                    nslots = 2 * (len(groups0) + len(groups1))
                    state = {"fi": 0, "si": 0}

                    def adv():
                        state["si"] += 1
                        tgt = state["si"] * nf // nslots
                        while state["fi"] < tgt:
                            filler[state["fi"]]()
                            state["fi"] += 1

                    for half, groups in ((0, groups0), (1, groups1)):
                        for g in groups:
                            if g[0] == "s":
                                emit_spatial_group(s, d, g[1], g[2], drip=adv)
                            else:
                                emit_temporal_group(s, d, g[1], g[2], drip=adv)
                            adv()
                        emit_norm_and_out(s, d, (half,))
                    while state["fi"] < nf:
                        filler[state["fi"]]()
                        state["fi"] += 1
                    del ds[s]

    nc.compile()
    return nc


def _get_nc(compute: str):
    if compute not in _CACHE:
        _CACHE[compute] = _build(compute)
    return _CACHE[compute]


def _np_dtype(compute: str):
    if compute == "f32":
        return np.float32
    import ml_dtypes

    return ml_dtypes.bfloat16


def _prep_in_maps(x, w_qkv, w_proj, b_proj, compute=None):
    dt = _np_dtype(compute or COMPUTE)
    x = np.asarray(x, dtype=np.float32).reshape(B, N, D)
    xT = np.ascontiguousarray(x.transpose(0, 2, 1)).astype(dt)
    wqkvT = np.ascontiguousarray(np.asarray(w_qkv, np.float32).T).astype(dt)
    wprojT = np.ascontiguousarray(np.asarray(w_proj, np.float32).T).astype(dt)
    bias = np.asarray(b_proj, np.float32).reshape(D, 1)

    mask = np.zeros((112, 112), np.float32)
    for g in range(7):
        mask[16 * g : 16 * (g + 1), 16 * g : 16 * (g + 1)] = 1.0
    mask = mask.astype(dt)

    return [
        {"xt": xT[b], "wqkvT": wqkvT, "wprojT": wprojT, "bias": bias, "mask": mask}
        for b in range(B)
    ]


def _postprocess(results):
    out = np.stack([r["outT"].T for r in results])
    return np.ascontiguousarray(out.reshape(B, F, P, D)).astype(np.float32)


def kernel(x, w_qkv, w_proj, b_proj):
    nc = _get_nc(COMPUTE)
    in_maps = _prep_in_maps(x, w_qkv, w_proj, b_proj)
    res = run_bass_kernel_spmd(nc, in_maps, core_ids=list(range(B)))
    return _postprocess(res.results)


# revision 40
# speedup vs baseline: 1.0113x; 1.0113x over previous
"""Trainium2 Bass kernel for factorized space-time attention.

Computation (per batch b of 8, one NeuronCore each):
  qkv = x @ w_qkv.T                      (3136, 2304)
  heads 0-5:  spatial attention over 196 patches within each of 16 frames
  heads 6-11: temporal attention over groups of 16 consecutive tokens
              (raw-reshape semantics of the reference)
  out = concat(head outputs) @ w_proj.T + b_proj

Strategy: data-parallel over batch (8 cores). All activations kept
feature-major ([d, n]) on chip so every matmul contraction runs over the
partition dim with no on-device transposes; x / weights are pre-transposed
host-side. V is produced token-major directly by flipping the projection
matmul orientation, with a ones column appended so the AV matmul emits the
softmax denominator in psum row 64 for free.

The kernel is software-pipelined at emission level: the QKV/V projection
matmuls for superblock s+1 are interleaved between the attention groups of
superblock s, so the in-order PE always has independent projection work
while Act/DVE process softmax (exp, reciprocal, normalize).  Softmax
normalization is batched: per group only reciprocals (into per-head-pair
collector rows) and raw-AV copies run; per superblock-half the collected
reciprocals are partition-broadcast by ones-matmuls and applied with wide
[64,392] muls, and head1 rows reach attnT partitions 64:128 via batched
DMA. Temporal masking runs on the otherwise idle Pool (gpsimd) engine.

All matmul operands are bf16 (4x faster than fp32 on the PE); accumulation
stays fp32 in PSUM.
"""

import sys

if "/opt/trn_rl_repo" not in sys.path:
    sys.path.append("/opt/trn_rl_repo")

import numpy as np

import concourse.bass as bass  # noqa: F401
import concourse.mybir as mybir
import concourse.tile as tile
from concourse import bacc
from concourse.bass_utils import run_bass_kernel_spmd

F32 = mybir.dt.float32
BF16 = mybir.dt.bfloat16
AF = mybir.ActivationFunctionType

B = 8
F = 16
P = 196
D = 768
NH = 12
HD = 64
N = F * P
E3 = 3 * D
SB = 784
NSB = N // SB
FPSB = SB // P
WPSB = SB // 112
SCALE = HD ** -0.5

COMPUTE = "bf16"

_CACHE = {}


def _build(compute: str, reps: int = 1):
    cdt = BF16 if compute == "bf16" else F32

    wb = 2

    nc = bacc.Bacc("TRN2", target_bir_lowering=False, debug=False, num_devices=B)

    xt_d = nc.dram_tensor("xt", (D, N), cdt, kind="ExternalInput")
    wqkv_d = nc.dram_tensor("wqkvT", (D, E3), cdt, kind="ExternalInput")
    wproj_d = nc.dram_tensor("wprojT", (D, D), cdt, kind="ExternalInput")
    bias_d = nc.dram_tensor("bias", (D, 1), F32, kind="ExternalInput")
    mask_d = nc.dram_tensor("mask", (112, 112), cdt, kind="ExternalInput")
    out_d = nc.dram_tensor("outT", (D, N), F32, kind="ExternalOutput")

    with tile.TileContext(nc) as tc:
        with (
            tc.tile_pool(name="const", bufs=1) as cpool,
            tc.tile_pool(name="work", bufs=1) as wpool,
            tc.tile_pool(name="small", bufs=4) as spool,
            tc.tile_pool(name="psum", bufs=2, space="PSUM") as ppool,
        ):
            wq = []
            for dc in range(6):
                t = cpool.tile([128, E3], cdt, tag=f"wq{dc}", name=f"wq{dc}")
                nc.sync.dma_start(t[:], wqkv_d.ap()[128 * dc : 128 * (dc + 1), :])
                wq.append(t)
            wp = []
            for dc in range(6):
                t = cpool.tile([128, D], cdt, tag=f"wp{dc}", name=f"wp{dc}")
                nc.sync.dma_start(t[:], wproj_d.ap()[128 * dc : 128 * (dc + 1), :])
                wp.append(t)
            bias_t = cpool.tile([128, 6], F32, tag="bias", name="bias_t")
            nc.sync.dma_start(
                bias_t[:], bias_d.ap().rearrange("(e p) one -> p (e one)", p=128)
            )
            mask2_t = cpool.tile([112, 224], cdt, tag="mask", name="mask2_t")
            nc.sync.dma_start(mask2_t[:, 0:112], mask_d.ap())
            nc.sync.dma_start(mask2_t[:, 112:224], mask_d.ap())
            zeros_col = cpool.tile([128, 1], F32, tag="zeros_c", name="zeros_col")
            nc.gpsimd.memset(zeros_col[:], 0.0)
            ones64 = cpool.tile([65, 64], cdt, tag="ones64", name="ones64")
            nc.gpsimd.memset(ones64[:], 1.0)

            cp_ctr = [0]

            def bulk_copy(dst, src):
                i = cp_ctr[0] % 2
                cp_ctr[0] += 1
                if i == 0:
                    nc.scalar.copy(dst, src)
                else:
                    nc.vector.tensor_copy(dst, src)

            # ---- per-superblock tile groups ----
            def alloc_sb(s):
                d = {"vs": {}, "vt": {}}
                d["xts"] = [
                    wpool.tile([128, SB], cdt, tag=f"xts{dc}", bufs=wb,
                               name=f"xts{dc}_{s}")
                    for dc in range(6)
                ]
                d["qkvt"] = [
                    wpool.tile([128, SB], cdt, tag=f"qkvt{ti}", bufs=wb,
                               name=f"qkvt{ti}_{s}")
                    for ti in range(12)
                ]
                d["attnT"] = [
                    wpool.tile([128, SB], cdt, tag=f"attnT{i}", bufs=wb,
                               name=f"attnT{i}_{s}")
                    for i in range(6)
                ]
                d["rcol"] = [
                    wpool.tile([65, 2 * SB], cdt, tag=f"rcol{i}", bufs=wb,
                               name=f"rcol{i}_{s}")
                    for i in range(6)
                ]
                d["hst"] = [
                    wpool.tile([64, 2 * SB], cdt, tag=f"hst{i}", bufs=1,
                               name=f"hst{i}_{s}")
                    for i in range(6)
                ]
                return d

            def emit_x_dma(s, d):
                so = SB * s
                for dc in range(6):
                    nc.sync.dma_start(
                        d["xts"][dc][:],
                        xt_d.ap()[128 * dc : 128 * (dc + 1), so : so + SB],
                    )

            def emit_qk_unit(s, d, ti, j):
                ps = ppool.tile([128, 392], F32, tag="mm", bufs=2,
                                name=f"ps_qk{s}_{ti}_{j}")
                for dc in range(6):
                    nc.tensor.matmul(
                        ps[:],
                        wq[dc][:, 128 * ti : 128 * (ti + 1)],
                        d["xts"][dc][:, 392 * j : 392 * (j + 1)],
                        start=(dc == 0),
                        stop=(dc == 5),
                    )
                bulk_copy(d["qkvt"][ti][:, 392 * j : 392 * (j + 1)], ps[:])

            def emit_v_unit(s, d, kind, idx):
                if kind == "s":
                    f, ci = divmod(idx, 2)
                    m0, msz = ((0, 128), (128, 68))[ci]
                    tok0, wcol0 = 196 * f + m0, 1536
                    vname = f"vs{idx}_{s}"
                else:
                    msz, tok0, wcol0 = 112, 112 * idx, 1920
                    vname = f"vt{idx}_{s}"
                vt_ = wpool.tile([msz, 390], cdt, tag=f"v{kind}{idx}", bufs=wb,
                                 name=vname)
                ps = ppool.tile([msz, 384], F32, tag="mm", bufs=2,
                                name=f"ps_{vname}")
                for dc in range(6):
                    nc.tensor.matmul(
                        ps[:],
                        d["xts"][dc][:, tok0 : tok0 + msz],
                        wq[dc][:, wcol0 : wcol0 + 384],
                        start=(dc == 0),
                        stop=(dc == 5),
                    )
                bulk_copy(
                    vt_.rearrange("p (h c) -> p h c", c=65)[:, :, 0:64],
                    ps.rearrange("p (h c) -> p h c", c=64),
                )
                nc.gpsimd.memset(
                    vt_.rearrange("p (h c) -> p h c", c=65)[:, :, 64:65], 1.0
                )
                d["vs" if kind == "s" else "vt"][idx] = vt_

            def emit_proj_units(s, d):
                units = [lambda s=s, d=d: emit_x_dma(s, d)]
                for ti in range(12):
                    for j in range(2):
                        units.append(
                            lambda s=s, d=d, ti=ti, j=j: emit_qk_unit(s, d, ti, j)
                        )
                for idx in range(2 * FPSB):
                    units.append(
                        lambda s=s, d=d, idx=idx: emit_v_unit(s, d, "s", idx)
                    )
                for idx in range(WPSB):
                    units.append(
                        lambda s=s, d=d, idx=idx: emit_v_unit(s, d, "t", idx)
                    )
                return units

            def emit_spatial_group(s, d, f, hp):
                fo = 196 * f
                qkvt, vs = d["qkvt"], d["vs"]
                qtile, ktile = qkvt[hp], qkvt[6 + hp]
                es_all = []
                for hi in range(2):
                    pb = 64 * hi
                    es = []
                    for ci, (m0, msz) in enumerate(((0, 128), (128, 68))):
                        ps_st = ppool.tile(
                            [msz, 196], F32, tag="st", bufs=3,
                            name=f"ps_st{s}_{f}_{hp}_{hi}_{ci}",
                        )
                        nc.tensor.matmul(
                            ps_st[:],
                            ktile[pb : pb + 64, fo + m0 : fo + m0 + msz],
                            qtile[pb : pb + 64, fo : fo + 196],
                            start=True,
                            stop=True,
                        )
                        e = spool.tile(
                            [msz, 196], cdt, tag="e", bufs=6,
                            name=f"e{s}_{f}_{hp}_{hi}_{ci}",
                        )
                        nc.scalar.activation(
                            e[:], ps_st[:], AF.Exp,
                            bias=zeros_col[:msz, :], scale=SCALE,
                        )
                        es.append(e)
                    es_all.append(es)
                av = ppool.tile([65, 392], F32, tag="av", bufs=3,
                                name=f"ps_sav{s}_{f}_{hp}")
                for hi in range(2):
                    h = 2 * hp + hi
                    for ci in range(2):
                        nc.tensor.matmul(
                            av[:, 196 * hi : 196 * hi + 196],
                            vs[2 * f + ci][:, 65 * h : 65 * h + 65],
                            es_all[hi][ci][:],
                            start=(ci == 0),
                            stop=(ci == 1),
                            skip_group_check=(hi == 1),
                        )
                with nc.allow_low_precision(reason="1/softmax-sum in cdt"):
                    nc.vector.reciprocal(
                        d["rcol"][hp][64:65].rearrange(
                            "p (h t) -> p h t", h=2
                        )[:, :, fo : fo + 196],
                        av[64:65, :].rearrange("p (h t) -> p h t", h=2),
                    )
                bulk_copy(
                    d["hst"][hp].rearrange("p (h t) -> p h t", h=2)[
                        :, :, fo : fo + 196
                    ],
                    av[0:64, :].rearrange("p (h t) -> p h t", h=2),
                )

            def emit_temporal_group(s, d, w, hp):
                wo = 112 * w
                qkvt, vt = d["qkvt"], d["vt"]
                ems = []
                for hi in range(2):
                    pb = 64 * hi
                    ps_st = ppool.tile(
                        [112, 112], F32, tag="st", bufs=3,
                        name=f"ps_tst{s}_{w}_{hp}_{hi}",
                    )
                    nc.tensor.matmul(
                        ps_st[:],
                        qkvt[9 + hp][pb : pb + 64, wo : wo + 112],
                        qkvt[3 + hp][pb : pb + 64, wo : wo + 112],
                        start=True,
                        stop=True,
                    )
                    e = spool.tile(
                        [112, 112], cdt, tag="e", bufs=6,
                        name=f"et{s}_{w}_{hp}_{hi}",
                    )
                    nc.scalar.activation(
                        e[:], ps_st[:], AF.Exp,
                        bias=zeros_col[:112], scale=SCALE,
                    )
                    em = spool.tile(
                        [112, 112], cdt, tag="e", bufs=6,
                        name=f"em{s}_{w}_{hp}_{hi}",
                    )
                    nc.gpsimd.tensor_mul(em[:], e[:], mask2_t[:, 0:112])
                    ems.append(em)
                av = ppool.tile([65, 224], F32, tag="av", bufs=3,
                                name=f"ps_tav{s}_{w}_{hp}")
                for hi in range(2):
                    h = 6 + 2 * hp + hi
                    nc.tensor.matmul(
                        av[:, 112 * hi : 112 * hi + 112],
                        vt[w][:, 65 * (h - 6) : 65 * (h - 6) + 65],
                        ems[hi][:],
                        start=True,
                        stop=True,
                        skip_group_check=(hi == 1),
                    )
                with nc.allow_low_precision(reason="1/softmax-sum in cdt"):
                    nc.vector.reciprocal(
                        d["rcol"][3 + hp][64:65].rearrange(
                            "p (h t) -> p h t", h=2
                        )[:, :, wo : wo + 112],
                        av[64:65, :].rearrange("p (h t) -> p h t", h=2),
                    )
                bulk_copy(
                    d["hst"][3 + hp].rearrange("p (h t) -> p h t", h=2)[
                        :, :, wo : wo + 112
                    ],
                    av[0:64, :].rearrange("p (h t) -> p h t", h=2),
                )

            def emit_norm_and_out(s, d, js):
                so = SB * s
                for j in js:
                    jo = 392 * j
                    for ai in range(6):
                        rc, at = d["rcol"][ai], d["attnT"][ai]
                        rb0 = ppool.tile(
                            [64, 392], F32, tag="mm", bufs=2,
                            name=f"ps_rb0{s}_{j}_{ai}",
                        )
                        nc.tensor.matmul(
                            rb0[:], ones64[64:65, 0:64],
                            rc[64:65, jo : jo + 392],
                            start=True, stop=True,
                        )
                        nc.vector.tensor_mul(
                            at[0:64, jo : jo + 392],
                            d["hst"][ai][:, jo : jo + 392], rb0[:],
                        )
                        rb1 = ppool.tile(
                            [64, 392], F32, tag="mm", bufs=2,
                            name=f"ps_rb1{s}_{j}_{ai}",
                        )
                        nc.tensor.matmul(
                            rb1[:], ones64[64:65, 0:64],
                            rc[64:65, SB + jo : SB + jo + 392],
                            start=True, stop=True,
                        )
                        nc.vector.tensor_mul(
                            d["hst"][ai][:, jo : jo + 392],
                            d["hst"][ai][:, SB + jo : SB + jo + 392], rb1[:],
                        )
                        nc.sync.dma_start(
                            at[64:128, jo : jo + 392],
                            d["hst"][ai][:, jo : jo + 392],
                        )
                    for ec in range(6):
                        ps = ppool.tile([128, 392], F32, tag="mm", bufs=2,
                                        name=f"ps_o{s}_{ec}_{j}")
                        for dc in range(6):
                            nc.tensor.matmul(
                                ps[:],
                                wp[dc][:, 128 * ec : 128 * (ec + 1)],
                                d["attnT"][dc][:, 392 * j : 392 * (j + 1)],
                                start=(dc == 0),
                                stop=(dc == 5),
                            )
                        ot = spool.tile([128, 392], F32, tag="ot",
                                        name=f"ot{s}_{ec}_{j}")
                        nc.scalar.activation(
                            ot[:], ps[:], AF.Identity,
                            bias=bias_t[:, ec : ec + 1], scale=1.0,
                        )
                        nc.sync.dma_start(
                            out_d.ap()[
                                128 * ec : 128 * (ec + 1),
                                so + 392 * j : so + 392 * (j + 1),
                            ],
                            ot[:],
                        )

            import contextlib

            rep_ctx = tc.For_i(0, reps, 1) if reps > 1 else contextlib.nullcontext()
            with rep_ctx:
                ds = {0: alloc_sb(0)}
                for u in emit_proj_units(0, ds[0]):
                    u()
                for s in range(NSB):
                    d = ds[s]
                    # first half covers attnT cols 0:392 (frames 0-1, windows
                    # 0-3); second half the rest
                    groups0 = [("s", f, hp) for f in (0, 1) for hp in range(3)]
                    groups0 += [("t", w, hp) for w in (0, 1, 2, 3)
                                for hp in range(3)]
                    groups1 = [("s", f, hp) for f in (2, 3) for hp in range(3)]
                    groups1 += [("t", w, hp) for w in (4, 5, 6)
                                for hp in range(3)]
                    if s + 1 < NSB:
                        ds[s + 1] = alloc_sb(s + 1)
                        filler = emit_proj_units(s + 1, ds[s + 1])
                    else:
                        filler = []
                    nf = len(filler)
                    ng = len(groups0) + len(groups1)
                    fi = 0
                    gi = 0
                    for half, groups in ((0, groups0), (1, groups1)):
                        for g in groups:
                            if g[0] == "s":
                                emit_spatial_group(s, d, g[1], g[2])
                            else:
                                emit_temporal_group(s, d, g[1], g[2])
                            gi += 1
                            tgt = gi * nf // ng
                            while fi < tgt:
                                filler[fi]()
                                fi += 1
                        emit_norm_and_out(s, d, (half,))
                    while fi < nf:
                        filler[fi]()
                        fi += 1
                    del ds[s]

    nc.compile()
    return nc


def _get_nc(compute: str):
    if compute not in _CACHE:
        _CACHE[compute] = _build(compute)
    return _CACHE[compute]


def _np_dtype(compute: str):
    if compute == "f32":
        return np.float32
    import ml_dtypes

    return ml_dtypes.bfloat16


def _prep_in_maps(x, w_qkv, w_proj, b_proj, compute=None):
    dt = _np_dtype(compute or COMPUTE)
    x = np.asarray(x, dtype=np.float32).reshape(B, N, D)
    xT = np.ascontiguousarray(x.transpose(0, 2, 1)).astype(dt)
    wqkvT = np.ascontiguousarray(np.asarray(w_qkv, np.float32).T).astype(dt)
    wprojT = np.ascontiguousarray(np.asarray(w_proj, np.float32).T).astype(dt)
    bias = np.asarray(b_proj, np.float32).reshape(D, 1)

    mask = np.zeros((112, 112), np.float32)
    for g in range(7):
        mask[16 * g : 16 * (g + 1), 16 * g : 16 * (g + 1)] = 1.0
    mask = mask.astype(dt)

    return [
        {"xt": xT[b], "wqkvT": wqkvT, "wprojT": wprojT, "bias": bias, "mask": mask}
        for b in range(B)
    ]


def _postprocess(results):
    out = np.stack([r["outT"].T for r in results])
    return np.ascontiguousarray(out.reshape(B, F, P, D)).astype(np.float32)


def kernel(x, w_qkv, w_proj, b_proj):
    nc = _get_nc(COMPUTE)
    in_maps = _prep_in_maps(x, w_qkv, w_proj, b_proj)
    res = run_bass_kernel_spmd(nc, in_maps, core_ids=list(range(B)))
    return _postprocess(res.results)


# revision 41
# speedup vs baseline: 1.0137x; 1.0023x over previous
"""Trainium2 Bass kernel for factorized space-time attention.

Computation (per batch b of 8, one NeuronCore each):
  qkv = x @ w_qkv.T                      (3136, 2304)
  heads 0-5:  spatial attention over 196 patches within each of 16 frames
  heads 6-11: temporal attention over groups of 16 consecutive tokens
              (raw-reshape semantics of the reference)
  out = concat(head outputs) @ w_proj.T + b_proj

Strategy: data-parallel over batch (8 cores). All activations kept
feature-major ([d, n]) on chip so every matmul contraction runs over the
partition dim with no on-device transposes; x / weights are pre-transposed
host-side. V is produced token-major directly by flipping the projection
matmul orientation, with a ones column appended so the AV matmul emits the
softmax denominator in psum row 64 for free.

The kernel is software-pipelined at emission level: the QKV/V projection
matmuls for superblock s+1 are interleaved between the attention groups of
superblock s, so the in-order PE always has independent projection work
while Act/DVE process softmax (exp, reciprocal, normalize).  Softmax
normalization is batched: per group only reciprocals (into per-head-pair
collector rows) and raw-AV copies run; per superblock-half the collected
reciprocals are partition-broadcast by ones-matmuls and applied with wide
[64,392] muls, and head1 rows reach attnT partitions 64:128 via batched
DMA. Temporal masking runs on the otherwise idle Pool (gpsimd) engine.

All matmul operands are bf16 (4x faster than fp32 on the PE); accumulation
stays fp32 in PSUM.
"""

import sys

if "/opt/trn_rl_repo" not in sys.path:
    sys.path.append("/opt/trn_rl_repo")

import numpy as np

import concourse.bass as bass  # noqa: F401
import concourse.mybir as mybir
import concourse.tile as tile
from concourse import bacc
from concourse.bass_utils import run_bass_kernel_spmd

F32 = mybir.dt.float32
BF16 = mybir.dt.bfloat16
AF = mybir.ActivationFunctionType

B = 8
F = 16
P = 196
D = 768
NH = 12
HD = 64
N = F * P
E3 = 3 * D
SB = 784
NSB = N // SB
FPSB = SB // P
WPSB = SB // 112
SCALE = HD ** -0.5

COMPUTE = "bf16"

_CACHE = {}


def _build(compute: str, reps: int = 1):
    cdt = BF16 if compute == "bf16" else F32

    wb = 2

    nc = bacc.Bacc("TRN2", target_bir_lowering=False, debug=False, num_devices=B)

    xt_d = nc.dram_tensor("xt", (D, N), cdt, kind="ExternalInput")
    wqkv_d = nc.dram_tensor("wqkvT", (D, E3), cdt, kind="ExternalInput")
    wproj_d = nc.dram_tensor("wprojT", (D, D), cdt, kind="ExternalInput")
    bias_d = nc.dram_tensor("bias", (D, 1), F32, kind="ExternalInput")
    mask_d = nc.dram_tensor("mask", (7, 224), cdt, kind="ExternalInput")
    out_d = nc.dram_tensor("outT", (D, N), F32, kind="ExternalOutput")

    with tile.TileContext(nc) as tc:
        with (
            tc.tile_pool(name="const", bufs=1) as cpool,
            tc.tile_pool(name="work", bufs=1) as wpool,
            tc.tile_pool(name="small", bufs=4) as spool,
            tc.tile_pool(name="psum", bufs=2, space="PSUM") as ppool,
        ):
            wq = []
            for dc in range(6):
                t = cpool.tile([128, E3], cdt, tag=f"wq{dc}", name=f"wq{dc}")
                nc.sync.dma_start(t[:], wqkv_d.ap()[128 * dc : 128 * (dc + 1), :])
                wq.append(t)
            wp = []
            for dc in range(6):
                t = cpool.tile([128, D], cdt, tag=f"wp{dc}", name=f"wp{dc}")
                nc.sync.dma_start(t[:], wproj_d.ap()[128 * dc : 128 * (dc + 1), :])
                wp.append(t)
            bias_t = cpool.tile([128, 6], F32, tag="bias", name="bias_t")
            nc.sync.dma_start(
                bias_t[:], bias_d.ap().rearrange("(e p) one -> p (e one)", p=128)
            )
            m7 = cpool.tile([7, 224], cdt, tag="mask", name="m7")
            nc.sync.dma_start(m7[:], mask_d.ap())
            zeros_col = cpool.tile([128, 1], F32, tag="zeros_c", name="zeros_col")
            nc.gpsimd.memset(zeros_col[:], 0.0)
            ones64 = cpool.tile([65, 64], cdt, tag="ones64", name="ones64")
            nc.gpsimd.memset(ones64[:], 1.0)

            cp_ctr = [0]

            def bulk_copy(dst, src):
                i = cp_ctr[0] % 2
                cp_ctr[0] += 1
                if i == 0:
                    nc.scalar.copy(dst, src)
                else:
                    nc.vector.tensor_copy(dst, src)

            # ---- per-superblock tile groups ----
            def alloc_sb(s):
                d = {"vs": {}, "vt": {}}
                d["xts"] = [
                    wpool.tile([128, SB], cdt, tag=f"xts{dc}", bufs=wb,
                               name=f"xts{dc}_{s}")
                    for dc in range(6)
                ]
                d["qkvt"] = [
                    wpool.tile([128, SB], cdt, tag=f"qkvt{ti}", bufs=wb,
                               name=f"qkvt{ti}_{s}")
                    for ti in range(12)
                ]
                d["attnT"] = [
                    wpool.tile([128, SB], cdt, tag=f"attnT{i}", bufs=wb,
                               name=f"attnT{i}_{s}")
                    for i in range(6)
                ]
                d["rcol"] = [
                    wpool.tile([65, 2 * SB], cdt, tag=f"rcol{i}", bufs=wb,
                               name=f"rcol{i}_{s}")
                    for i in range(6)
                ]
                d["hst"] = [
                    wpool.tile([64, 2 * SB], cdt, tag=f"hst{i}", bufs=1,
                               name=f"hst{i}_{s}")
                    for i in range(6)
                ]
                return d

            def emit_x_dma(s, d):
                so = SB * s
                for dc in range(6):
                    nc.sync.dma_start(
                        d["xts"][dc][:],
                        xt_d.ap()[128 * dc : 128 * (dc + 1), so : so + SB],
                    )

            def emit_qk_unit(s, d, ti, j):
                ps = ppool.tile([128, 392], F32, tag="mm", bufs=2,
                                name=f"ps_qk{s}_{ti}_{j}")
                for dc in range(6):
                    nc.tensor.matmul(
                        ps[:],
                        wq[dc][:, 128 * ti : 128 * (ti + 1)],
                        d["xts"][dc][:, 392 * j : 392 * (j + 1)],
                        start=(dc == 0),
                        stop=(dc == 5),
                    )
                bulk_copy(d["qkvt"][ti][:, 392 * j : 392 * (j + 1)], ps[:])

            def emit_v_unit(s, d, kind, idx):
                if kind == "s":
                    f, ci = divmod(idx, 2)
                    m0, msz = ((0, 128), (128, 68))[ci]
                    tok0, wcol0 = 196 * f + m0, 1536
                    vname = f"vs{idx}_{s}"
                else:
                    msz, tok0, wcol0 = 112, 112 * idx, 1920
                    vname = f"vt{idx}_{s}"
                vt_ = wpool.tile([msz, 390], cdt, tag=f"v{kind}{idx}", bufs=wb,
                                 name=vname)
                ps = ppool.tile([msz, 384], F32, tag="mm", bufs=2,
                                name=f"ps_{vname}")
                for dc in range(6):
                    nc.tensor.matmul(
                        ps[:],
                        d["xts"][dc][:, tok0 : tok0 + msz],
                        wq[dc][:, wcol0 : wcol0 + 384],
                        start=(dc == 0),
                        stop=(dc == 5),
                    )
                bulk_copy(
                    vt_.rearrange("p (h c) -> p h c", c=65)[:, :, 0:64],
                    ps.rearrange("p (h c) -> p h c", c=64),
                )
                nc.gpsimd.memset(
                    vt_.rearrange("p (h c) -> p h c", c=65)[:, :, 64:65], 1.0
                )
                d["vs" if kind == "s" else "vt"][idx] = vt_

            def emit_proj_units(s, d):
                units = [lambda s=s, d=d: emit_x_dma(s, d)]
                for ti in range(12):
                    for j in range(2):
                        units.append(
                            lambda s=s, d=d, ti=ti, j=j: emit_qk_unit(s, d, ti, j)
                        )
                for idx in range(2 * FPSB):
                    units.append(
                        lambda s=s, d=d, idx=idx: emit_v_unit(s, d, "s", idx)
                    )
                for idx in range(WPSB):
                    units.append(
                        lambda s=s, d=d, idx=idx: emit_v_unit(s, d, "t", idx)
                    )
                return units

            def emit_spatial_group(s, d, f, hp):
                fo = 196 * f
                qkvt, vs = d["qkvt"], d["vs"]
                qtile, ktile = qkvt[hp], qkvt[6 + hp]
                es_all = []
                for hi in range(2):
                    pb = 64 * hi
                    es = []
                    for ci, (m0, msz) in enumerate(((0, 128), (128, 68))):
                        ps_st = ppool.tile(
                            [msz, 196], F32, tag="st", bufs=3,
                            name=f"ps_st{s}_{f}_{hp}_{hi}_{ci}",
                        )
                        nc.tensor.matmul(
                            ps_st[:],
                            ktile[pb : pb + 64, fo + m0 : fo + m0 + msz],
                            qtile[pb : pb + 64, fo : fo + 196],
                            start=True,
                            stop=True,
                        )
                        e = spool.tile(
                            [msz, 196], cdt, tag="e", bufs=6,
                            name=f"e{s}_{f}_{hp}_{hi}_{ci}",
                        )
                        nc.scalar.activation(
                            e[:], ps_st[:], AF.Exp,
                            bias=zeros_col[:msz, :], scale=SCALE,
                        )
                        es.append(e)
                    es_all.append(es)
                av = ppool.tile([65, 392], F32, tag="av", bufs=3,
                                name=f"ps_sav{s}_{f}_{hp}")
                for hi in range(2):
                    h = 2 * hp + hi
                    for ci in range(2):
                        nc.tensor.matmul(
                            av[:, 196 * hi : 196 * hi + 196],
                            vs[2 * f + ci][:, 65 * h : 65 * h + 65],
                            es_all[hi][ci][:],
                            start=(ci == 0),
                            stop=(ci == 1),
                            skip_group_check=(hi == 1),
                        )
                with nc.allow_low_precision(reason="1/softmax-sum in cdt"):
                    nc.vector.reciprocal(
                        d["rcol"][hp][64:65].rearrange(
                            "p (h t) -> p h t", h=2
                        )[:, :, fo : fo + 196],
                        av[64:65, :].rearrange("p (h t) -> p h t", h=2),
                    )
                bulk_copy(
                    d["hst"][hp].rearrange("p (h t) -> p h t", h=2)[
                        :, :, fo : fo + 196
                    ],
                    av[0:64, :].rearrange("p (h t) -> p h t", h=2),
                )

            def emit_temporal_group(s, d, w, hp):
                wo = 112 * w
                qkvt, vt = d["qkvt"], d["vt"]
                ems = []
                for hi in range(2):
                    pb = 64 * hi
                    ps_st = ppool.tile(
                        [112, 112], F32, tag="st", bufs=3,
                        name=f"ps_tst{s}_{w}_{hp}_{hi}",
                    )
                    nc.tensor.matmul(
                        ps_st[:],
                        qkvt[9 + hp][pb : pb + 64, wo : wo + 112],
                        qkvt[3 + hp][pb : pb + 64, wo : wo + 112],
                        start=True,
                        stop=False,
                    )
                    # rank-7 in-block bias (+240 pre-scale = +30 post-scale);
                    # e^30 scales in-block terms, cancelled by normalization
                    nc.tensor.matmul(
                        ps_st[:],
                        m7[:, 0:112],
                        m7[:, 112:224],
                        start=False,
                        stop=True,
                    )
                    em = spool.tile(
                        [112, 112], cdt, tag="e", bufs=6,
                        name=f"et{s}_{w}_{hp}_{hi}",
                    )
                    nc.scalar.activation(
                        em[:], ps_st[:], AF.Exp,
                        bias=zeros_col[:112], scale=SCALE,
                    )
                    ems.append(em)
                av = ppool.tile([65, 224], F32, tag="av", bufs=3,
                                name=f"ps_tav{s}_{w}_{hp}")
                for hi in range(2):
                    h = 6 + 2 * hp + hi
                    nc.tensor.matmul(
                        av[:, 112 * hi : 112 * hi + 112],
                        vt[w][:, 65 * (h - 6) : 65 * (h - 6) + 65],
                        ems[hi][:],
                        start=True,
                        stop=True,
                        skip_group_check=(hi == 1),
                    )
                with nc.allow_low_precision(reason="1/softmax-sum in cdt"):
                    nc.vector.reciprocal(
                        d["rcol"][3 + hp][64:65].rearrange(
                            "p (h t) -> p h t", h=2
                        )[:, :, wo : wo + 112],
                        av[64:65, :].rearrange("p (h t) -> p h t", h=2),
                    )
                bulk_copy(
                    d["hst"][3 + hp].rearrange("p (h t) -> p h t", h=2)[
                        :, :, wo : wo + 112
                    ],
                    av[0:64, :].rearrange("p (h t) -> p h t", h=2),
                )

            def emit_norm_and_out(s, d, js):
                so = SB * s
                for j in js:
                    jo = 392 * j
                    for ai in range(6):
                        rc, at = d["rcol"][ai], d["attnT"][ai]
                        rb0 = ppool.tile(
                            [64, 392], F32, tag="mm", bufs=2,
                            name=f"ps_rb0{s}_{j}_{ai}",
                        )
                        nc.tensor.matmul(
                            rb0[:], ones64[64:65, 0:64],
                            rc[64:65, jo : jo + 392],
                            start=True, stop=True,
                        )
                        nc.vector.tensor_mul(
                            at[0:64, jo : jo + 392],
                            d["hst"][ai][:, jo : jo + 392], rb0[:],
                        )
                        rb1 = ppool.tile(
                            [64, 392], F32, tag="mm", bufs=2,
                            name=f"ps_rb1{s}_{j}_{ai}",
                        )
                        nc.tensor.matmul(
                            rb1[:], ones64[64:65, 0:64],
                            rc[64:65, SB + jo : SB + jo + 392],
                            start=True, stop=True,
                        )
                        nc.vector.tensor_mul(
                            d["hst"][ai][:, jo : jo + 392],
                            d["hst"][ai][:, SB + jo : SB + jo + 392], rb1[:],
                        )
                        nc.sync.dma_start(
                            at[64:128, jo : jo + 392],
                            d["hst"][ai][:, jo : jo + 392],
                        )
                    for ec in range(6):
                        ps = ppool.tile([128, 392], F32, tag="mm", bufs=2,
                                        name=f"ps_o{s}_{ec}_{j}")
                        for dc in range(6):
                            nc.tensor.matmul(
                                ps[:],
                                wp[dc][:, 128 * ec : 128 * (ec + 1)],
                                d["attnT"][dc][:, 392 * j : 392 * (j + 1)],
                                start=(dc == 0),
                                stop=(dc == 5),
                            )
                        ot = spool.tile([128, 392], F32, tag="ot",
                                        name=f"ot{s}_{ec}_{j}")
                        nc.scalar.activation(
                            ot[:], ps[:], AF.Identity,
                            bias=bias_t[:, ec : ec + 1], scale=1.0,
                        )
                        nc.sync.dma_start(
                            out_d.ap()[
                                128 * ec : 128 * (ec + 1),
                                so + 392 * j : so + 392 * (j + 1),
                            ],
                            ot[:],
                        )

            import contextlib

            rep_ctx = tc.For_i(0, reps, 1) if reps > 1 else contextlib.nullcontext()
            with rep_ctx:
                ds = {0: alloc_sb(0)}
                for u in emit_proj_units(0, ds[0]):
                    u()
                for s in range(NSB):
                    d = ds[s]
                    # first half covers attnT cols 0:392 (frames 0-1, windows
                    # 0-3); second half the rest
                    groups0 = [("s", f, hp) for f in (0, 1) for hp in range(3)]
                    groups0 += [("t", w, hp) for w in (0, 1, 2, 3)
                                for hp in range(3)]
                    groups1 = [("s", f, hp) for f in (2, 3) for hp in range(3)]
                    groups1 += [("t", w, hp) for w in (4, 5, 6)
                                for hp in range(3)]
                    if s + 1 < NSB:
                        ds[s + 1] = alloc_sb(s + 1)
                        filler = emit_proj_units(s + 1, ds[s + 1])
                    else:
                        filler = []
                    nf = len(filler)
                    ng = len(groups0) + len(groups1)
                    fi = 0
                    gi = 0
                    for half, groups in ((0, groups0), (1, groups1)):
                        for g in groups:
                            if g[0] == "s":
                                emit_spatial_group(s, d, g[1], g[2])
                            else:
                                emit_temporal_group(s, d, g[1], g[2])
                            gi += 1
                            tgt = gi * nf // ng
                            while fi < tgt:
                                filler[fi]()
                                fi += 1
                        emit_norm_and_out(s, d, (half,))
                    while fi < nf:
                        filler[fi]()
                        fi += 1
                    del ds[s]

    nc.compile()
    return nc


def _get_nc(compute: str):
    if compute not in _CACHE:
        _CACHE[compute] = _build(compute)
    return _CACHE[compute]


def _np_dtype(compute: str):
    if compute == "f32":
        return np.float32
    import ml_dtypes

    return ml_dtypes.bfloat16


def _prep_in_maps(x, w_qkv, w_proj, b_proj, compute=None):
    dt = _np_dtype(compute or COMPUTE)
    x = np.asarray(x, dtype=np.float32).reshape(B, N, D)
    xT = np.ascontiguousarray(x.transpose(0, 2, 1)).astype(dt)
    wqkvT = np.ascontiguousarray(np.asarray(w_qkv, np.float32).T).astype(dt)
    wprojT = np.ascontiguousarray(np.asarray(w_proj, np.float32).T).astype(dt)
    bias = np.asarray(b_proj, np.float32).reshape(D, 1)

    mask = np.zeros((7, 224), np.float32)
    for g in range(7):
        mask[g, 16 * g : 16 * (g + 1)] = 240.0
        mask[g, 112 + 16 * g : 112 + 16 * (g + 1)] = 1.0
    mask = mask.astype(dt)

    return [
        {"xt": xT[b], "wqkvT": wqkvT, "wprojT": wprojT, "bias": bias, "mask": mask}
        for b in range(B)
    ]


def _postprocess(results):
    out = np.stack([r["outT"].T for r in results])
    return np.ascontiguousarray(out.reshape(B, F, P, D)).astype(np.float32)


def kernel(x, w_qkv, w_proj, b_proj):
    nc = _get_nc(COMPUTE)
    in_maps = _prep_in_maps(x, w_qkv, w_proj, b_proj)
    res = run_bass_kernel_spmd(nc, in_maps, core_ids=list(range(B)))
    return _postprocess(res.results)


# revision 44
# speedup vs baseline: 1.0382x; 1.0242x over previous
"""Trainium2 Bass kernel for factorized space-time attention.

Computation (per batch b of 8, one NeuronCore each):
  qkv = x @ w_qkv.T                      (3136, 2304)
  heads 0-5:  spatial attention over 196 patches within each of 16 frames
  heads 6-11: temporal attention over groups of 16 consecutive tokens
              (raw-reshape semantics of the reference)
  out = concat(head outputs) @ w_proj.T + b_proj

Strategy: data-parallel over batch (8 cores). All activations kept
feature-major ([d, n]) on chip so every matmul contraction runs over the
partition dim with no on-device transposes; x / weights are pre-transposed
host-side. V is produced token-major directly by flipping the projection
matmul orientation, with a ones column appended so the AV matmul emits the
softmax denominator in psum row 64 for free.

The kernel is software-pipelined at emission level: the QKV/V projection
matmuls for superblock s+1 are interleaved between the attention groups of
superblock s, so the in-order PE always has independent projection work
while Act/DVE process softmax (exp, reciprocal, normalize).  Softmax
normalization is batched: per group only reciprocals (into per-head-pair
collector rows) and raw-AV copies run; per superblock-half the collected
reciprocals are partition-broadcast by ones-matmuls and applied with wide
[64,392] muls, and head1 rows reach attnT partitions 64:128 via batched
DMA. Temporal masking runs on the otherwise idle Pool (gpsimd) engine.

All matmul operands are bf16 (4x faster than fp32 on the PE); accumulation
stays fp32 in PSUM.
"""

import sys

if "/opt/trn_rl_repo" not in sys.path:
    sys.path.append("/opt/trn_rl_repo")

import numpy as np

import concourse.bass as bass  # noqa: F401
import concourse.mybir as mybir
import concourse.tile as tile
from concourse import bacc
from concourse.bass_utils import run_bass_kernel_spmd

F32 = mybir.dt.float32
BF16 = mybir.dt.bfloat16
AF = mybir.ActivationFunctionType

B = 8
F = 16
P = 196
D = 768
NH = 12
HD = 64
N = F * P
E3 = 3 * D
SB = 784
NSB = N // SB
FPSB = SB // P
WPSB = SB // 112
SCALE = HD ** -0.5

COMPUTE = "bf16"

_CACHE = {}


def _build(compute: str, reps: int = 1):
    cdt = BF16 if compute == "bf16" else F32

    wb = 2

    nc = bacc.Bacc("TRN2", target_bir_lowering=False, debug=False, num_devices=B)

    xt_d = nc.dram_tensor("xt", (D, N), cdt, kind="ExternalInput")
    wqkv_d = nc.dram_tensor("wqkvT", (D, E3), cdt, kind="ExternalInput")
    wproj_d = nc.dram_tensor("wprojT", (D, D), cdt, kind="ExternalInput")
    bias_d = nc.dram_tensor("bias", (D, 1), F32, kind="ExternalInput")
    mask_d = nc.dram_tensor("mask", (112, 112), cdt, kind="ExternalInput")
    out_d = nc.dram_tensor("outT", (D, N), F32, kind="ExternalOutput")

    with tile.TileContext(nc) as tc:
        with (
            tc.tile_pool(name="const", bufs=1) as cpool,
            tc.tile_pool(name="work", bufs=1) as wpool,
            tc.tile_pool(name="small", bufs=4) as spool,
            tc.tile_pool(name="psum", bufs=2, space="PSUM") as ppool,
        ):
            wq = []
            for dc in range(6):
                t = cpool.tile([128, E3], cdt, tag=f"wq{dc}", name=f"wq{dc}")
                nc.sync.dma_start(t[:], wqkv_d.ap()[128 * dc : 128 * (dc + 1), :])
                wq.append(t)
            wp = []
            for dc in range(6):
                t = cpool.tile([128, D], cdt, tag=f"wp{dc}", name=f"wp{dc}")
                nc.sync.dma_start(t[:], wproj_d.ap()[128 * dc : 128 * (dc + 1), :])
                wp.append(t)
            bias_t = cpool.tile([128, 6], F32, tag="bias", name="bias_t")
            nc.sync.dma_start(
                bias_t[:], bias_d.ap().rearrange("(e p) one -> p (e one)", p=128)
            )
            mask2_t = cpool.tile([112, 224], cdt, tag="mask", name="mask2_t")
            nc.sync.dma_start(mask2_t[:, 0:112], mask_d.ap())
            nc.sync.dma_start(mask2_t[:, 112:224], mask_d.ap())
            zeros_col = cpool.tile([128, 1], F32, tag="zeros_c", name="zeros_col")
            nc.gpsimd.memset(zeros_col[:], 0.0)
            ones64 = cpool.tile([65, 64], cdt, tag="ones64", name="ones64")
            nc.gpsimd.memset(ones64[:], 1.0)

            cp_ctr = [0]

            def bulk_copy(dst, src):
                i = cp_ctr[0] % 2
                cp_ctr[0] += 1
                if i == 0:
                    nc.scalar.copy(dst, src)
                else:
                    nc.vector.tensor_copy(dst, src)

            # ---- per-superblock tile groups ----
            def alloc_sb(s):
                d = {"vs": {}, "vt": {}}
                d["xts"] = [
                    wpool.tile([128, SB], cdt, tag=f"xts{dc}", bufs=wb,
                               name=f"xts{dc}_{s}")
                    for dc in range(6)
                ]
                d["qkvt"] = [
                    wpool.tile([128, SB], cdt, tag=f"qkvt{ti}", bufs=wb,
                               name=f"qkvt{ti}_{s}")
                    for ti in range(12)
                ]
                d["attnT"] = [
                    wpool.tile([128, SB], cdt, tag=f"attnT{i}", bufs=wb,
                               name=f"attnT{i}_{s}")
                    for i in range(6)
                ]
                d["rcol"] = [
                    wpool.tile([65, 2 * SB], cdt, tag=f"rcol{i}", bufs=wb,
                               name=f"rcol{i}_{s}")
                    for i in range(6)
                ]
                d["hst"] = [
                    wpool.tile([64, 2 * SB], cdt, tag=f"hst{i}", bufs=1,
                               name=f"hst{i}_{s}")
                    for i in range(6)
                ]
                return d

            def emit_x_dma(s, d):
                so = SB * s
                for dc in range(6):
                    nc.sync.dma_start(
                        d["xts"][dc][:],
                        xt_d.ap()[128 * dc : 128 * (dc + 1), so : so + SB],
                    )

            def emit_qk_unit(s, d, ti, j):
                ps = ppool.tile([128, 392], F32, tag="mm", bufs=2,
                                name=f"ps_qk{s}_{ti}_{j}")
                for dc in range(6):
                    nc.tensor.matmul(
                        ps[:],
                        wq[dc][:, 128 * ti : 128 * (ti + 1)],
                        d["xts"][dc][:, 392 * j : 392 * (j + 1)],
                        start=(dc == 0),
                        stop=(dc == 5),
                    )
                bulk_copy(d["qkvt"][ti][:, 392 * j : 392 * (j + 1)], ps[:])

            def emit_v_unit(s, d, kind, idx):
                if kind == "s":
                    f, ci = divmod(idx, 2)
                    m0, msz = ((0, 128), (128, 68))[ci]
                    tok0, wcol0 = 196 * f + m0, 1536
                    vname = f"vs{idx}_{s}"
                else:
                    msz, tok0, wcol0 = 112, 112 * idx, 1920
                    vname = f"vt{idx}_{s}"
                vt_ = wpool.tile([msz, 390], cdt, tag=f"v{kind}{idx}", bufs=wb,
                                 name=vname)
                ps = ppool.tile([msz, 384], F32, tag="mm", bufs=2,
                                name=f"ps_{vname}")
                for dc in range(6):
                    nc.tensor.matmul(
                        ps[:],
                        d["xts"][dc][:, tok0 : tok0 + msz],
                        wq[dc][:, wcol0 : wcol0 + 384],
                        start=(dc == 0),
                        stop=(dc == 5),
                    )
                bulk_copy(
                    vt_.rearrange("p (h c) -> p h c", c=65)[:, :, 0:64],
                    ps.rearrange("p (h c) -> p h c", c=64),
                )
                nc.gpsimd.memset(
                    vt_.rearrange("p (h c) -> p h c", c=65)[:, :, 64:65], 1.0
                )
                d["vs" if kind == "s" else "vt"][idx] = vt_

            def emit_proj_units(s, d):
                units = [lambda s=s, d=d: emit_x_dma(s, d)]
                for ti in range(12):
                    for j in range(2):
                        units.append(
                            lambda s=s, d=d, ti=ti, j=j: emit_qk_unit(s, d, ti, j)
                        )
                for idx in range(2 * FPSB):
                    units.append(
                        lambda s=s, d=d, idx=idx: emit_v_unit(s, d, "s", idx)
                    )
                for idx in range(WPSB):
                    units.append(
                        lambda s=s, d=d, idx=idx: emit_v_unit(s, d, "t", idx)
                    )
                return units

            def emit_spatial_group(s, d, f, hp):
                fo = 196 * f
                qkvt, vs = d["qkvt"], d["vs"]
                qtile, ktile = qkvt[hp], qkvt[6 + hp]
                es_all = []
                for hi in range(2):
                    pb = 64 * hi
                    es = []
                    for ci, (m0, msz) in enumerate(((0, 128), (128, 68))):
                        ps_st = ppool.tile(
                            [msz, 196], F32, tag="st", bufs=4,
                            name=f"ps_st{s}_{f}_{hp}_{hi}_{ci}",
                        )
                        nc.tensor.matmul(
                            ps_st[:],
                            ktile[pb : pb + 64, fo + m0 : fo + m0 + msz],
                            qtile[pb : pb + 64, fo : fo + 196],
                            start=True,
                            stop=True,
                        )
                        e = spool.tile(
                            [msz, 196], cdt, tag="e", bufs=6,
                            name=f"e{s}_{f}_{hp}_{hi}_{ci}",
                        )
                        nc.scalar.activation(
                            e[:], ps_st[:], AF.Exp,
                            bias=zeros_col[:msz, :], scale=SCALE,
                        )
                        es.append(e)
                    es_all.append(es)
                av = ppool.tile([65, 392], F32, tag="av", bufs=2,
                                name=f"ps_sav{s}_{f}_{hp}")
                for hi in range(2):
                    h = 2 * hp + hi
                    for ci in range(2):
                        nc.tensor.matmul(
                            av[:, 196 * hi : 196 * hi + 196],
                            vs[2 * f + ci][:, 65 * h : 65 * h + 65],
                            es_all[hi][ci][:],
                            start=(ci == 0),
                            stop=(ci == 1),
                            skip_group_check=(hi == 1),
                        )
                with nc.allow_low_precision(reason="1/softmax-sum in cdt"):
                    nc.vector.reciprocal(
                        d["rcol"][hp][64:65].rearrange(
                            "p (h t) -> p h t", h=2
                        )[:, :, fo : fo + 196],
                        av[64:65, :].rearrange("p (h t) -> p h t", h=2),
                    )
                bulk_copy(
                    d["hst"][hp].rearrange("p (h t) -> p h t", h=2)[
                        :, :, fo : fo + 196
                    ],
                    av[0:64, :].rearrange("p (h t) -> p h t", h=2),
                )

            def emit_temporal_group(s, d, w, hp):
                wo = 112 * w
                qkvt, vt = d["qkvt"], d["vt"]
                ems = []
                for hi in range(2):
                    pb = 64 * hi
                    ps_st = ppool.tile(
                        [112, 112], F32, tag="st", bufs=4,
                        name=f"ps_tst{s}_{w}_{hp}_{hi}",
                    )
                    nc.tensor.matmul(
                        ps_st[:],
                        qkvt[9 + hp][pb : pb + 64, wo : wo + 112],
                        qkvt[3 + hp][pb : pb + 64, wo : wo + 112],
                        start=True,
                        stop=True,
                    )
                    e = spool.tile(
                        [112, 112], cdt, tag="e", bufs=6,
                        name=f"et{s}_{w}_{hp}_{hi}",
                    )
                    nc.scalar.activation(
                        e[:], ps_st[:], AF.Exp,
                        bias=zeros_col[:112], scale=SCALE,
                    )
                    em = spool.tile(
                        [112, 112], cdt, tag="e", bufs=6,
                        name=f"em{s}_{w}_{hp}_{hi}",
                    )
                    nc.gpsimd.tensor_mul(em[:], e[:], mask2_t[:, 0:112])
                    ems.append(em)
                av = ppool.tile([65, 224], F32, tag="av", bufs=2,
                                name=f"ps_tav{s}_{w}_{hp}")
                for hi in range(2):
                    h = 6 + 2 * hp + hi
                    nc.tensor.matmul(
                        av[:, 112 * hi : 112 * hi + 112],
                        vt[w][:, 65 * (h - 6) : 65 * (h - 6) + 65],
                        ems[hi][:],
                        start=True,
                        stop=True,
                        skip_group_check=(hi == 1),
                    )
                with nc.allow_low_precision(reason="1/softmax-sum in cdt"):
                    nc.vector.reciprocal(
                        d["rcol"][3 + hp][64:65].rearrange(
                            "p (h t) -> p h t", h=2
                        )[:, :, wo : wo + 112],
                        av[64:65, :].rearrange("p (h t) -> p h t", h=2),
                    )
                bulk_copy(
                    d["hst"][3 + hp].rearrange("p (h t) -> p h t", h=2)[
                        :, :, wo : wo + 112
                    ],
                    av[0:64, :].rearrange("p (h t) -> p h t", h=2),
                )

            def emit_norm_and_out(s, d, js):
                so = SB * s
                for j in js:
                    jo = 392 * j
                    for ai in range(6):
                        rc, at = d["rcol"][ai], d["attnT"][ai]
                        rb0 = ppool.tile(
                            [64, 392], F32, tag="mm", bufs=2,
                            name=f"ps_rb0{s}_{j}_{ai}",
                        )
                        nc.tensor.matmul(
                            rb0[:], ones64[64:65, 0:64],
                            rc[64:65, jo : jo + 392],
                            start=True, stop=True,
                        )
                        nc.vector.tensor_mul(
                            at[0:64, jo : jo + 392],
                            d["hst"][ai][:, jo : jo + 392], rb0[:],
                        )
                        rb1 = ppool.tile(
                            [64, 392], F32, tag="mm", bufs=2,
                            name=f"ps_rb1{s}_{j}_{ai}",
                        )
                        nc.tensor.matmul(
                            rb1[:], ones64[64:65, 0:64],
                            rc[64:65, SB + jo : SB + jo + 392],
                            start=True, stop=True,
                        )
                        nc.vector.tensor_mul(
                            d["hst"][ai][:, jo : jo + 392],
                            d["hst"][ai][:, SB + jo : SB + jo + 392], rb1[:],
                        )
                        nc.sync.dma_start(
                            at[64:128, jo : jo + 392],
                            d["hst"][ai][:, jo : jo + 392],
                        )
                    for ec in range(6):
                        ps = ppool.tile([128, 392], F32, tag="mm", bufs=2,
                                        name=f"ps_o{s}_{ec}_{j}")
                        for dc in range(6):
                            nc.tensor.matmul(
                                ps[:],
                                wp[dc][:, 128 * ec : 128 * (ec + 1)],
                                d["attnT"][dc][:, 392 * j : 392 * (j + 1)],
                                start=(dc == 0),
                                stop=(dc == 5),
                            )
                        ot = spool.tile([128, 392], F32, tag="ot",
                                        name=f"ot{s}_{ec}_{j}")
                        nc.scalar.activation(
                            ot[:], ps[:], AF.Identity,
                            bias=bias_t[:, ec : ec + 1], scale=1.0,
                        )
                        nc.sync.dma_start(
                            out_d.ap()[
                                128 * ec : 128 * (ec + 1),
                                so + 392 * j : so + 392 * (j + 1),
                            ],
                            ot[:],
                        )

            import contextlib

            rep_ctx = tc.For_i(0, reps, 1) if reps > 1 else contextlib.nullcontext()
            with rep_ctx:
                ds = {0: alloc_sb(0)}
                for u in emit_proj_units(0, ds[0]):
                    u()
                for s in range(NSB):
                    d = ds[s]
                    # first half covers attnT cols 0:392 (frames 0-1, windows
                    # 0-3); second half the rest
                    groups0 = [("s", f, hp) for f in (0, 1) for hp in range(3)]
                    groups0 += [("t", w, hp) for w in (0, 1, 2, 3)
                                for hp in range(3)]
                    groups1 = [("s", f, hp) for f in (2, 3) for hp in range(3)]
                    groups1 += [("t", w, hp) for w in (4, 5, 6)
                                for hp in range(3)]
                    if s + 1 < NSB:
                        ds[s + 1] = alloc_sb(s + 1)
                        filler = emit_proj_units(s + 1, ds[s + 1])
                    else:
                        filler = []
                    nf = len(filler)
                    ng = len(groups0) + len(groups1)
                    fi = 0
                    gi = 0
                    for half, groups in ((0, groups0), (1, groups1)):
                        for g in groups:
                            if g[0] == "s":
                                emit_spatial_group(s, d, g[1], g[2])
                            else:
                                emit_temporal_group(s, d, g[1], g[2])
                            gi += 1
                            tgt = gi * nf // ng
                            while fi < tgt:
                                filler[fi]()
                                fi += 1
                        emit_norm_and_out(s, d, (half,))
                    while fi < nf:
                        filler[fi]()
                        fi += 1
                    del ds[s]

    nc.compile()
    return nc


def _get_nc(compute: str):
    if compute not in _CACHE:
        _CACHE[compute] = _build(compute)
    return _CACHE[compute]


def _np_dtype(compute: str):
    if compute == "f32":
        return np.float32
    import ml_dtypes

    return ml_dtypes.bfloat16


def _prep_in_maps(x, w_qkv, w_proj, b_proj, compute=None):
    dt = _np_dtype(compute or COMPUTE)
    x = np.asarray(x, dtype=np.float32).reshape(B, N, D)
    xT = np.ascontiguousarray(x.transpose(0, 2, 1)).astype(dt)
    wqkvT = np.ascontiguousarray(np.asarray(w_qkv, np.float32).T).astype(dt)
    wprojT = np.ascontiguousarray(np.asarray(w_proj, np.float32).T).astype(dt)
    bias = np.asarray(b_proj, np.float32).reshape(D, 1)

    mask = np.zeros((112, 112), np.float32)
    for g in range(7):
        mask[16 * g : 16 * (g + 1), 16 * g : 16 * (g + 1)] = 1.0
    mask = mask.astype(dt)

    return [
        {"xt": xT[b], "wqkvT": wqkvT, "wprojT": wprojT, "bias": bias, "mask": mask}
        for b in range(B)
    ]


def _postprocess(results):
    out = np.stack([r["outT"].T for r in results])
    return np.ascontiguousarray(out.reshape(B, F, P, D)).astype(np.float32)


def kernel(x, w_qkv, w_proj, b_proj):
    nc = _get_nc(COMPUTE)
    in_maps = _prep_in_maps(x, w_qkv, w_proj, b_proj)
    res = run_bass_kernel_spmd(nc, in_maps, core_ids=list(range(B)))
    return _postprocess(res.results)
